# revision 24
# baseline (speedup 1.0000x reference)
"""Trainium2 Bass kernel for nn_Decoder_63720134804045.

Ragged-sequence compaction: the reference zeroes every heap node whose
existence mask is false, and with P(slash)=2/50 only ~2% of the
B*S*31 node-rows are live. The host computes the existence mask and a
compaction index (pure indexing, as the baseline already did for its
one-hot/mask layouts); the device runs the full model math -- embedding
gathers, the three DxD GEMMs + LN (folded affine, rank-1 mean
correction in PSUM), neighbor-leaf GEMM over the depth-restricted slot
union, and softmax -- over the compacted rows only. Data-parallel over
batch: 8 cores x 4 batches, padded to a common row budget R.
"""
import sys
sys.path.insert(0, '/opt/trn_rl_repo')
from contextlib import ExitStack

import numpy as np

import concourse.bass as bass
import concourse.tile as tile
from concourse import bacc, mybir
from concourse._compat import with_exitstack
from concourse.bass_utils import run_bass_kernel_spmd
from concourse.masks import make_identity

F32 = mybir.dt.float32
F32R = mybir.dt.float32r
I32 = mybir.dt.int32
AF = mybir.ActivationFunctionType
ALU = mybir.AluOpType

B, S, D, V = 32, 64, 768, 50
MAXD, LC = 5, 3
NN = 31                 # heap nodes
NSLOT = 63
NCORES = 8
BL = B // NCORES        # 4 local batches
KC = D // 128           # 6 feature chunks
EPS = 1e-5
NOFF = 5                # neighbor shift offsets [-3,-2,-1,1,2]
OFFS = [-3, -2, -1, 1, 2]
LSLOT = 15              # leaf slots per neighbor block in the padded layout

_CACHE = {}
RSTD_GPSIMD = False


def _build_nc(geom, loop_n=None, nbody=1):
    """geom = (R, Lr, KD): row budget, leaf-row budget, leaf K dim.

    nbody > 1 emits several stage-interleaved copies of the body per loop
    iteration: engines overlap across bodies (the For_i back-edge is an
    all-engine barrier) and same-table Act ops batch, amortizing
    activation-table reloads.
    """
    R, Lr, KD = geom
    nc = bacc.Bacc("TRN2", target_bir_lowering=False, debug=False,
                   num_devices=NCORES)
    dt = nc.dram_tensor
    nblk = R // 128
    kcl = KD // 128
    ins = dict(
        memC=dt("memC", [128, KC * R], F32, kind="ExternalInput"),
        idxg=dt("idxg", [128, nblk], I32, kind="ExternalInput"),
        W1=dt("W1", [D, D], F32R, kind="ExternalInput"),
        W2=dt("W2", [D, D], F32R, kind="ExternalInput"),
        W3=dt("W3", [D, D], F32R, kind="ExternalInput"),
        Wout=dt("Wout", [D, V], F32R, kind="ExternalInput"),
        biases=dt("biases", [128, 4 * KC], F32, kind="ExternalInput"),
        vrow=dt("vrow", [1, 2 * D], F32R, kind="ExternalInput"),
        femb=dt("femb", [20000, D], F32, kind="ExternalInput"),
    )
    if Lr:
        ins.update(
            lembp=dt("lembp", [V + 1, 32], F32, kind="ExternalInput"),
            lidx=dt("lidx", [128, kcl], I32, kind="ExternalInput"),
            leafWs=dt("leafWs", [KD, D], F32R, kind="ExternalInput"),
            eye4=dt("eye4", [128, 32], F32, kind="ExternalInput"),
        )
    out_d = dt("out", [R, V], F32, kind="ExternalOutput")
    aps = {k: v.ap() for k, v in ins.items()}
    with tile.TileContext(nc) as tc:
        with tc.tile_pool(name="pw", bufs=1) as pw:
            Wsb = _load_weights(tc, pw, aps, geom)
            if loop_n is None:
                _kernel_body(tc, aps, out_d.ap(), Wsb, geom, nbody)
            else:
                with tc.For_i(0, loop_n, 1):
                    _kernel_body(tc, aps, out_d.ap(), Wsb, geom, nbody)
    nc.compile()
    return nc


def _load_weights(tc, pw, ins, geom):
    """Input-constant SBUF state, loaded once (outside the timing loop)."""
    R, Lr, KD = geom
    nc = tc.nc
    Wsb = {}
    for wname in ("W1", "W2", "W3"):
        for kc in range(KC):
            t_ = pw.tile([128, D], F32R, tag=f"{wname}_{kc}")
            nc.sync.dma_start(t_[:], ins[wname][kc * 128:(kc + 1) * 128, :])
            Wsb[(wname, kc)] = t_
    for kc in range(KC):
        t_ = pw.tile([128, V], F32R, tag=f"wout_{kc}")
        nc.sync.dma_start(t_[:], ins["Wout"][kc * 128:(kc + 1) * 128, :])
        Wsb[("Wout", kc)] = t_
    if Lr:
        for kc in range(KD // 128):
            t_ = pw.tile([128, D], F32R, tag=f"lw_{kc}")
            nc.sync.dma_start(t_[:], ins["leafWs"][kc * 128:(kc + 1) * 128, :])
            Wsb[("LW", kc)] = t_
        eye4 = pw.tile([128, 32], F32)
        nc.sync.dma_start(eye4[:], ins["eye4"][:])
        Wsb["eye4"] = eye4
    bias_sb = pw.tile([128, 4 * KC], F32)
    nc.sync.dma_start(bias_sb[:], ins["biases"][:])
    Wsb["bias"] = bias_sb
    vrow_sb = pw.tile([1, 2 * D], F32R)
    nc.sync.dma_start(vrow_sb[:], ins["vrow"][:])
    Wsb["vrow"] = vrow_sb
    ident = pw.tile([128, 128], F32)
    make_identity(nc, ident[:])
    Wsb["ident"] = ident
    ones_c = pw.tile([128, 1], F32R)
    ones_cf = pw.tile([128, 1], F32)
    nc.vector.memset(ones_cf[:], 1.0)
    nc.vector.tensor_copy(ones_c[:], ones_cf[:])
    Wsb["ones_c"] = ones_c
    ones_rr = pw.tile([1, 128], F32R)
    nc.vector.memset(ones_rr[:].bitcast(F32), 1.0)
    Wsb["ones_rr"] = ones_rr
    eps_sb = pw.tile([1, 1], F32)
    nc.vector.memset(eps_sb[:], EPS)
    Wsb["eps"] = eps_sb
    return Wsb


@with_exitstack
def _kernel_body(ctx: ExitStack, tc: tile.TileContext, ins, out_d, Wsb, geom,
                 nbody=1):
    R, Lr, KD = geom
    NW = R
    nblk = R // 128
    nb2 = 2 * nbody
    nc = tc.nc
    p_io = ctx.enter_context(tc.tile_pool(name="p_io", bufs=max(2, nbody)))
    p_gth = ctx.enter_context(tc.tile_pool(name="p_gth", bufs=nb2))
    p_act = ctx.enter_context(tc.tile_pool(name="p_act", bufs=nbody))
    p_sm = ctx.enter_context(tc.tile_pool(name="p_sm", bufs=nb2))
    p_tg = ctx.enter_context(tc.tile_pool(name="p_tg", bufs=3))
    p_osb = ctx.enter_context(tc.tile_pool(name="p_osb", bufs=nb2))
    ps_mm = ctx.enter_context(tc.tile_pool(name="ps_mm", bufs=4, space="PSUM"))
    ps_tr = ctx.enter_context(tc.tile_pool(name="ps_tr", bufs=2, space="PSUM"))
    ps_st = ctx.enter_context(tc.tile_pool(name="ps_st", bufs=2, space="PSUM"))

    bias_sb = Wsb["bias"]
    ident = Wsb["ident"]
    st = [dict() for _ in range(nbody)]   # per-body live tiles

    def s_in(b):
        addb = p_io.tile([128, KC * NW], F32, tag="addb", name=f"addb_{b}")
        nc.sync.dma_start(addb[:], ins["memC"][:])
        idx_sb = p_io.tile([128, nblk], I32, tag="idxg", name=f"idxg_{b}")
        nc.sync.dma_start(idx_sb[:], ins["idxg"][:])
        st[b]["addb"], st[b]["idx"] = addb, idx_sb
        if Lr:
            kcl = KD // 128
            lidx_sb = p_io.tile([128, kcl], I32, tag="lidx", name=f"lidx_{b}")
            nc.sync.dma_start(lidx_sb[:], ins["lidx"][:])
            st[b]["lidx"] = lidx_sb

    def s_leaf(b):
        """OL^T [D, Lr] for the tail rows, added into addb."""
        if not Lr:
            return
        kcl = KD // 128
        addb, lidx_sb = st[b]["addb"], st[b]["lidx"]
        lv = p_io.tile([128, kcl * Lr], F32, tag="lv", name=f"lv_{b}")
        for kc in range(kcl):
            gl = p_gth.tile([128, 32], F32, tag="glemb", name=f"gl_{b}_{kc}")
            nc.gpsimd.indirect_dma_start(
                out=gl[:], out_offset=None, in_=ins["lembp"][:],
                in_offset=bass.IndirectOffsetOnAxis(
                    ap=lidx_sb[:, kc:kc + 1], axis=0))
            ptr = ps_tr.tile([128, 512], F32, space="PSUM", tag="ptr",
                             name=f"lptr_{b}_{kc}")
            for j in range(4):
                nc.tensor.matmul(
                    ptr[32 * j:32 * j + 32, 0:Lr],
                    gl[Lr * j:Lr * j + Lr, 0:32],
                    Wsb["eye4"][32 * j:32 * j + 32, 0:Lr],
                    start=True, stop=True, tile_position=(32 * j, 32 * j))
            nc.scalar.activation(lv[:, kc * Lr:(kc + 1) * Lr].bitcast(F32R),
                                 ptr[:, 0:Lr], AF.Identity)
        for mc in range(KC):
            pol = ps_mm.tile([128, NW], F32, space="PSUM", tag="pmm",
                             name=f"pol_{b}_{mc}")
            for kc in range(kcl):
                nc.tensor.matmul(
                    pol[:, 0:Lr], Wsb[("LW", kc)][:, mc * 128:(mc + 1) * 128],
                    lv[:, kc * Lr:(kc + 1) * Lr].bitcast(F32R),
                    start=(kc == 0), stop=(kc == kcl - 1))
            olsb = p_gth.tile([128, Lr], F32, tag="olsb", name=f"ol_{b}_{mc}")
            nc.scalar.activation(
                olsb[:], pol[:, 0:Lr], AF.Identity,
                bias=bias_sb[:, 3 * KC + mc:3 * KC + mc + 1])
            tl = slice(mc * NW + NW - Lr, (mc + 1) * NW)
            nc.vector.tensor_add(addb[:, tl], addb[:, tl], olsb[:])

    def s_gather(b):
        """Gather + transpose embeddings (feature-major embT)."""
        embT = p_act.tile([128, KC * NW], F32, tag="embT", name=f"embT_{b}")
        idx_sb = st[b]["idx"]
        for j in range(nblk):
            gth = p_gth.tile([128, D], F32, tag="gth", name=f"gth_{b}_{j}")
            nc.gpsimd.indirect_dma_start(
                out=gth[:], out_offset=None, in_=ins["femb"][:],
                in_offset=bass.IndirectOffsetOnAxis(
                    ap=idx_sb[:, j:j + 1], axis=0))
            for grp, glen in ((0, 4), (4, 2)):
                ptr = ps_tr.tile([128, 512], F32, space="PSUM", tag="ptr",
                                 name=f"ptr_{b}_{j}_{grp}")
                for ki in range(glen):
                    nc.tensor.transpose(
                        ptr[:, ki * 128:(ki + 1) * 128],
                        gth[:, (grp + ki) * 128:(grp + ki + 1) * 128],
                        ident[:])
                dst = (embT[:].rearrange("p (k w) -> p k w", w=NW)
                       [:, grp:grp + glen, j * 128:(j + 1) * 128])
                nc.vector.tensor_copy(dst.bitcast(F32R), ptr[:, 0:glen * 128])
        st[b]["embT"] = embT

    def fused_layer(b, src, dst, wname, bias_col, A_=None, m_=None, vcol=None):
        """dst = gelu(W^T src [*A - v (x) m] + b); LN applied in psum domain."""
        def mm_group(mc):
            pl = ps_mm.tile([128, NW], F32, space="PSUM", tag="pmm",
                            name=f"pl_{b}_{wname}_{mc}")
            for kc in range(KC):
                nc.tensor.matmul(
                    pl[:], Wsb[(wname, kc)][:, mc * 128:(mc + 1) * 128],
                    src[:, kc * NW:(kc + 1) * NW].bitcast(F32R),
                    start=(kc == 0), stop=(kc == KC - 1 and vcol is None))
            if vcol is not None:
                nc.tensor.matmul(
                    pl[:], Wsb["vrow"][0:1, vcol * D + mc * 128:vcol * D + (mc + 1) * 128],
                    m_[:].bitcast(F32R), start=False, stop=True)
            return pl

        def finish(mc, pl, Asb):
            sl = slice(mc * NW, (mc + 1) * NW)
            if Asb is None:
                nc.scalar.activation(
                    dst[:, sl].bitcast(F32R), pl[:], AF.Gelu,
                    bias=bias_sb[:, bias_col * KC + mc:bias_col * KC + mc + 1])
            else:
                tgc = p_tg.tile([128, NW], F32, tag="tg",
                                name=f"tg_{b}_{wname}_{mc}")
                nc.vector.tensor_mul(tgc[:], pl[:], Asb[:])
                nc.scalar.activation(
                    dst[:, sl].bitcast(F32R), tgc[:], AF.Gelu,
                    bias=bias_sb[:, bias_col * KC + mc:bias_col * KC + mc + 1])

        if vcol is None:
            for mc in range(KC):
                finish(mc, mm_group(mc), None)
            return
        pls = [mm_group(mc) for mc in range(3)]
        pA = ps_mm.tile([128, NW], F32, space="PSUM", tag="pmm",
                        name=f"pA_{b}_{wname}")
        nc.tensor.matmul(pA[:], Wsb["ones_rr"][0:1, :], A_[:].bitcast(F32R),
                         start=True, stop=True)
        Asb = p_sm.tile([128, NW], F32, tag="Asb", name=f"As_{b}_{wname}")
        nc.scalar.activation(Asb[:], pA[:], AF.Identity)
        for i in range(3):
            finish(i, pls[i], Asb)
            pls.append(mm_group(3 + i))
        for i in range(3, KC):
            finish(i, pls[i], Asb)

    def ln_stats(b, src, sq, tagsfx):
        """LN stats: (A_ = rstd row, m_ = mean row), both [1, NW]."""
        for mc in range(KC):
            nc.vector.tensor_mul(sq[:, mc * NW:(mc + 1) * NW].bitcast(F32R),
                                 src[:, mc * NW:(mc + 1) * NW],
                                 src[:, mc * NW:(mc + 1) * NW])
        pss = ps_st.tile([1, NW], F32, space="PSUM", tag="pst",
                         name=f"pss_{b}_{tagsfx}")
        for kc in range(KC):
            nc.tensor.matmul(pss[0:1, :], Wsb["ones_c"][:, 0:1],
                             src[:, kc * NW:(kc + 1) * NW].bitcast(F32R),
                             start=(kc == 0), stop=(kc == KC - 1))
        psq = ps_st.tile([1, NW], F32, space="PSUM", tag="pst",
                         name=f"psq_{b}_{tagsfx}")
        for kc in range(KC):
            nc.tensor.matmul(psq[0:1, :], Wsb["ones_c"][:, 0:1],
                             sq[:, kc * NW:(kc + 1) * NW].bitcast(F32R),
                             start=(kc == 0), stop=(kc == KC - 1))
        m = p_sm.tile([1, NW], F32, tag="m", name=f"m_{b}_{tagsfx}")
        nc.vector.tensor_scalar(out=m[:].bitcast(F32R), in0=pss[0:1, :],
                                scalar1=1.0 / D, scalar2=None, op0=ALU.mult)
        msq = p_sm.tile([1, NW], F32, tag="msq", name=f"msq_{b}_{tagsfx}",
                        bufs=nbody)
        nc.vector.tensor_mul(msq[:], m[:], m[:])
        v = p_sm.tile([1, NW], F32, tag="v", name=f"v_{b}_{tagsfx}",
                      bufs=nbody)
        nc.vector.scalar_tensor_tensor(out=v[:], in0=psq[0:1, :], scalar=1.0 / D,
                                       in1=msq[:], op0=ALU.mult, op1=ALU.subtract)
        A_ = p_sm.tile([1, NW], F32, tag="A", name=f"A_{b}_{tagsfx}")
        with nc.allow_low_precision(reason="fp32r rounding of LN rstd"):
            sd = p_sm.tile([1, NW], F32, tag="sd", name=f"sd_{b}_{tagsfx}",
                           bufs=nbody)
            nc.scalar.activation(sd[:], v[:], AF.Sqrt,
                                 bias=Wsb["eps"][0:1, 0:1])
            nc.vector.reciprocal(A_[:].bitcast(F32R), sd[:])
        return A_, m

    def s_w1(b):
        h = p_act.tile([128, KC * NW], F32, tag="h", name=f"h_{b}")
        fused_layer(b, st[b]["embT"], h, "W1", 0)
        addb = st[b]["addb"]
        for mc in range(KC):
            sl = slice(mc * NW, (mc + 1) * NW)
            nc.vector.tensor_add(h[:, sl].bitcast(F32R), h[:, sl], addb[:, sl])
        st[b]["h"] = h

    def s_ln1(b):
        # sq scratch reuses the embT ring: embT is dead once W1's matmuls read it
        sq = p_act.tile([128, KC * NW], F32, tag="embT", name=f"sq1_{b}")
        st[b]["A1"], st[b]["m1"] = ln_stats(b, st[b]["h"], sq, "a")

    def s_w2(b):
        x2 = p_act.tile([128, KC * NW], F32, tag="x2", name=f"x2_{b}")
        fused_layer(b, st[b]["h"], x2, "W2", 1,
                    A_=st[b]["A1"], m_=st[b]["m1"], vcol=0)
        st[b]["x2"] = x2

    def s_ln2(b):
        sq = p_act.tile([128, KC * NW], F32, tag="embT", name=f"sq2_{b}")
        st[b]["A2"], st[b]["m2"] = ln_stats(b, st[b]["x2"], sq, "b")

    def s_w3(b):
        # x3 reuses the h ring: h is dead once W2's matmuls read it
        x3 = p_act.tile([128, KC * NW], F32, tag="h", name=f"x3_{b}")
        fused_layer(b, st[b]["x2"], x3, "W3", 2,
                    A_=st[b]["A2"], m_=st[b]["m2"], vcol=1)
        st[b]["x3"] = x3

    def s_out(b):
        po = ps_mm.tile([V, NW], F32, space="PSUM", tag="pmm", name=f"po_{b}")
        x3 = st[b]["x3"]
        for kc in range(KC):
            nc.tensor.matmul(po[:], Wsb[("Wout", kc)][:],
                             x3[:, kc * NW:(kc + 1) * NW].bitcast(F32R),
                             start=(kc == 0), stop=(kc == KC - 1))
        eT = p_act.tile([V, NW], F32, tag="eT", name=f"eT_{b}")
        nc.scalar.activation(eT[:], po[:], AF.Exp)
        for j in range(nblk):
            pt = ps_st.tile([128, V], F32, space="PSUM", tag="pst",
                            name=f"pt_{b}_{j}")
            nc.tensor.transpose(pt[:], eT[0:V, j * 128:(j + 1) * 128],
                                ident[0:V, 0:V])
            ssum = p_sm.tile([128, 1], F32, tag="ssum", name=f"ss_{b}_{j}")
            nc.vector.reduce_sum(ssum[:], pt[:], axis=mybir.AxisListType.X)
            rm = p_sm.tile([128, 1], F32, tag="rm", name=f"rm_{b}_{j}")
            nc.vector.reciprocal(rm[:], ssum[:])
            osb = p_osb.tile([128, V], F32, tag="osb", name=f"osb_{b}_{j}")
            nc.vector.tensor_scalar(out=osb[:], in0=pt[:], scalar1=rm[:],
                                    scalar2=None, op0=ALU.mult)
            nc.sync.dma_start(out_d[j * 128:(j + 1) * 128, :], osb[:])

    stages = [s_in, s_leaf, s_gather, s_w1, s_ln1, s_w2, s_ln2, s_w3, s_out]
    for stage in stages:
        for b in range(nbody):
            stage(b)


def _host_prep(inputs):
    """Pure index/layout prep: existence mask, compaction plan, weight
    folding. Returns (geom, in_maps, scatter) for the device run."""
    mem = np.asarray(inputs["memory"], np.float32)
    seqlen = np.asarray(inputs["seq_length"])
    tgt = np.asarray(inputs["tgt"])
    fidx = np.asarray(inputs["feat_idx"])
    femb = np.ascontiguousarray(np.asarray(inputs["feat_embs"], np.float32))
    W1 = np.ascontiguousarray(np.asarray(inputs["W1"], np.float32))
    ln_g = np.asarray(inputs["ln_g"], np.float32)
    ln_b = np.asarray(inputs["ln_b"], np.float32)
    W2 = np.asarray(inputs["W2"], np.float32)
    W3 = np.asarray(inputs["W3"], np.float32)
    b1 = np.asarray(inputs["b1"], np.float32)
    b2 = np.asarray(inputs["b2"], np.float32)
    b3 = np.asarray(inputs["b3"], np.float32)
    Wout = np.ascontiguousarray(np.asarray(inputs["Wout"], np.float32))
    lemb = np.ascontiguousarray(np.asarray(inputs["leaf_emb"], np.float32))
    lW = np.asarray(inputs["leaf_W"], np.float32)
    lb = np.asarray(inputs["leaf_b"], np.float32)

    W2f = np.ascontiguousarray(ln_g[:, None] * W2)
    W3f = np.ascontiguousarray(ln_g[:, None] * W3)
    b2f = (b2 + ln_b @ W2).astype(np.float32)
    b3f = (b3 + ln_b @ W3).astype(np.float32)

    tok_valid = np.arange(S)[None, :] < seqlen[:, None]
    is_slash = (tgt == 0) | (tgt == 1)
    ex = np.zeros((B, S, NN), bool)
    ex[:, :, 0] = tok_valid
    for i in range(1, NN):
        p = (i - 1) // 2
        ex[:, :, i] = ex[:, :, p] & is_slash[:, :, p]

    # compaction: per core, live rows; d>0 rows at the tail
    depth_of = np.zeros(NN, np.int64)
    for d in range(MAXD):
        depth_of[2 ** d - 1:2 ** (d + 1) - 1] = d
    rows_c, tails_c = [], []
    for c in range(NCORES):
        bsl = ex[c * BL:(c + 1) * BL]          # [BL,S,NN]
        bb, ss, nn_ = np.nonzero(bsl)
        dd = depth_of[nn_]
        order = np.argsort(dd > 0, kind="stable")
        bb, ss, nn_, dd = bb[order], ss[order], nn_[order], dd[order]
        head = [(int(b_), int(s_), int(n_)) for b_, s_, n_, d_ in
                zip(bb, ss, nn_, dd) if d_ == 0]
        tail = [(int(b_), int(s_), int(n_), int(d_)) for b_, s_, n_, d_ in
                zip(bb, ss, nn_, dd) if d_ > 0]
        rows_c.append(head)
        tails_c.append(tail)

    maxlive = max(len(h) + len(t) for h, t in zip(rows_c, tails_c))
    maxtail = max(len(t) for t in tails_c)
    maxd_live = max((t[3] for tl in tails_c for t in tl), default=0)
    Lr = 32 if maxtail else 0
    assert maxtail <= Lr, f"leaf budget overflow: {maxtail}"
    R = max(256, -(-maxlive // 128) * 128)
    assert maxlive + (1 if Lr else 0) * 0 <= R and R - Lr >= maxlive - maxtail

    # leaf slot union across live depths: (off n, leaf slot l) l < 2^(d-1)
    maxcnt = 2 ** (maxd_live - 1) if maxd_live else 0
    slots = [(n, l) for n in range(NOFF) for l in range(maxcnt)]
    while len(slots) % 4:
        slots.append(None)
    KD = len(slots) * 32
    geom = (R, Lr, KD)

    biases = np.stack([b1.reshape(KC, 128), b2f.reshape(KC, 128),
                       b3f.reshape(KC, 128), lb.reshape(KC, 128)])
    biases_sb = np.ascontiguousarray(biases.reshape(4 * KC, 128).T)
    vrow = np.concatenate([-W2f.sum(0), -W3f.sum(0)]).reshape(1, 2 * D).astype(np.float32)
    shared = dict(W1=W1, W2=W2f, W3=W3f, Wout=Wout, biases=biases_sb,
                  vrow=vrow, femb=femb)
    if Lr:
        lembp = np.concatenate([lemb, np.zeros((1, 32), np.float32)])
        # leafW rows for slot (n,l): flat rows ((n*LSLOT)+l)*32 ... +32
        lWs = np.zeros((KD, D), np.float32)
        for i, sl_ in enumerate(slots):
            if sl_ is None:
                continue
            n, l = sl_
            r0 = (n * LSLOT + l) * 32
            lWs[i * 32:(i + 1) * 32] = lW[r0:r0 + 32]
        shared.update(lembp=lembp, leafWs=np.ascontiguousarray(lWs),
                      eye4=np.ascontiguousarray(
                          np.tile(np.eye(32, dtype=np.float32), (4, 1))))

    in_maps, scatter = [], []
    tgt_p = np.pad(tgt, ((0, 0), (LC, LC), (0, 0)))          # [B,S+6,NN-ish]
    ex_p = np.pad(ex, ((0, 0), (LC, LC), (0, 0)))
    for c in range(NCORES):
        head, tail = rows_c[c], tails_c[c]
        n_h, n_t = len(head), len(tail)
        rows = list(head) + [(0, 0, 0)] * (R - Lr - n_h) if Lr else list(head)
        if Lr:
            rows += [(b_, s_, n_) for b_, s_, n_, _ in tail]
            rows += [(0, 0, 0)] * (Lr - n_t)
        else:
            rows += [(0, 0, 0)] * (R - n_h)
        assert len(rows) == R
        ridx = np.array([fidx[c * BL + b_, s_, n_] for b_, s_, n_ in rows],
                        np.int32)
        idxg = np.ascontiguousarray(ridx.reshape(R // 128, 128).T)
        memC_rows = np.zeros((R, D), np.float32)
        for i, (b_, s_, n_) in enumerate(rows):
            if i < n_h or (Lr and R - Lr <= i < R - Lr + n_t):
                memC_rows[i] = mem[c * BL + b_, s_]
        memC = np.ascontiguousarray(
            memC_rows.T.reshape(KC, 128, R).transpose(1, 0, 2)
            .reshape(128, KC * R))
        imap = dict(memC=memC, idxg=idxg, **shared)
        if Lr:
            # labels for tail leaf-row j, slot (n,l): depth d row at (b,s):
            # neighbor token s+off, tree slot a+l with a=2^(d-1)-1; masked ->
            # row V (zeros). Mask = ex at that node & valid l < cnt.
            lab = np.full((len(slots), Lr), V, np.int32)
            for j, (b_, s_, n_, d_) in enumerate(tail):
                a, cnt = 2 ** (d_ - 1) - 1, 2 ** (d_ - 1)
                gb = c * BL + b_
                for i, sl_ in enumerate(slots):
                    if sl_ is None:
                        continue
                    n_off, l = sl_
                    if l >= cnt:
                        continue
                    sp = s_ + LC + OFFS[n_off]
                    if ex_p[gb, sp, a + l]:
                        lab[i, j] = tgt_p[gb, sp, a + l]
            # gather order: chunk kc covers slots 4kc..4kc+4; partition
            # p = 32*slot_local + l
            kcl = KD // 128
            lidx = np.zeros((128, kcl), np.int32)
            for kc in range(kcl):
                for jloc in range(4):
                    lidx[32 * jloc:32 * jloc + 32, kc] = lab[4 * kc + jloc]
            imap.update(lidx=np.ascontiguousarray(lidx))
        in_maps.append(imap)
        scatter.append((rows, n_h, n_t))
    return geom, in_maps, scatter


def kernel(**inputs):
    geom, in_maps, scatter = _host_prep(inputs)
    if geom not in _CACHE:
        _CACHE[geom] = _build_nc(geom)
    nc = _CACHE[geom]
    res = run_bass_kernel_spmd(nc, in_maps, core_ids=list(range(NCORES)))
    R, Lr, _ = geom
    out = np.zeros((B, S, NSLOT, V), np.float32)
    for c in range(NCORES):
        dev = res.results[c]["out"]                      # [R, V]
        rows, n_h, n_t = scatter[c]
        for i in range(n_h):
            b_, s_, n_ = rows[i]
            out[c * BL + b_, s_, n_] = dev[i]
        for j in range(n_t):
            i = R - Lr + j
            b_, s_, n_ = rows[i]
            out[c * BL + b_, s_, n_] = dev[i]
    return out


# revision 29
# speedup vs baseline: 8.1309x; 8.1309x over previous
"""Trainium2 Bass kernel for nn_Decoder_63720134804045.

Ragged-sequence compaction: the reference zeroes every heap node whose
existence mask is false, and with P(slash)=2/50 only ~2% of the
B*S*31 node-rows are live. The host computes the existence mask and a
compaction index (pure indexing, as the baseline already did for its
one-hot/mask layouts); the device runs the full model math -- embedding
gathers, the three DxD GEMMs + LN (folded affine, rank-1 mean
correction in PSUM), neighbor-leaf GEMM over the depth-restricted slot
union, and softmax -- over the compacted rows only. Data-parallel over
batch: 8 cores x 4 batches, padded to a common row budget R.
"""
import sys
sys.path.insert(0, '/opt/trn_rl_repo')
from contextlib import ExitStack

import numpy as np

import concourse.bass as bass
import concourse.tile as tile
from concourse import bacc, mybir
from concourse._compat import with_exitstack
from concourse.bass_utils import run_bass_kernel_spmd
from concourse.masks import make_identity

F32 = mybir.dt.float32
F32R = mybir.dt.float32r
I32 = mybir.dt.int32
AF = mybir.ActivationFunctionType
ALU = mybir.AluOpType

B, S, D, V = 32, 64, 768, 50
MAXD, LC = 5, 3
NN = 31                 # heap nodes
NSLOT = 63
NCORES = 8
BL = B // NCORES        # 4 local batches
KC = D // 128           # 6 feature chunks
EPS = 1e-5
NOFF = 5                # neighbor shift offsets [-3,-2,-1,1,2]
OFFS = [-3, -2, -1, 1, 2]
LSLOT = 15              # leaf slots per neighbor block in the padded layout

_CACHE = {}
RSTD_GPSIMD = False


def _build_nc(geom, loop_n=None, nbody=1):
    """geom = (R, Lr, KD): row budget, leaf-row budget, leaf K dim.

    nbody > 1 emits several stage-interleaved copies of the body per loop
    iteration: engines overlap across bodies (the For_i back-edge is an
    all-engine barrier) and same-table Act ops batch, amortizing
    activation-table reloads.
    """
    R, Lr, KD = geom
    nc = bacc.Bacc("TRN2", target_bir_lowering=False, debug=False,
                   num_devices=NCORES)
    dt = nc.dram_tensor
    nblk = R // 128
    kcl = KD // 128
    ins = dict(
        memC=dt("memC", [128, KC * R], F32, kind="ExternalInput"),
        idxg=dt("idxg", [128, nblk], I32, kind="ExternalInput"),
        W1=dt("W1", [D, D], F32R, kind="ExternalInput"),
        W2=dt("W2", [D, D], F32R, kind="ExternalInput"),
        W3=dt("W3", [D, D], F32R, kind="ExternalInput"),
        Wout=dt("Wout", [D, V], F32R, kind="ExternalInput"),
        biases=dt("biases", [128, 4 * KC], F32, kind="ExternalInput"),
        vrow=dt("vrow", [1, 2 * D], F32R, kind="ExternalInput"),
        femb=dt("femb", [20000, D], F32, kind="ExternalInput"),
    )
    if Lr:
        ins.update(
            lembp=dt("lembp", [V + 1, 32], F32, kind="ExternalInput"),
            lidx=dt("lidx", [128, kcl], I32, kind="ExternalInput"),
            leafWs=dt("leafWs", [KD, D], F32R, kind="ExternalInput"),
            eye4=dt("eye4", [128, 32], F32, kind="ExternalInput"),
        )
    out_d = dt("out", [R, V], F32, kind="ExternalOutput")
    aps = {k: v.ap() for k, v in ins.items()}
    with tile.TileContext(nc) as tc:
        with tc.tile_pool(name="pw", bufs=1) as pw:
            Wsb = _load_weights(tc, pw, aps, geom)
            if loop_n is None:
                _kernel_body(tc, aps, out_d.ap(), Wsb, geom, nbody)
            else:
                with tc.For_i(0, loop_n, 1):
                    _kernel_body(tc, aps, out_d.ap(), Wsb, geom, nbody)
    nc.compile()
    return nc


def _load_weights(tc, pw, ins, geom):
    """Input-constant SBUF state, loaded once (outside the timing loop)."""
    R, Lr, KD = geom
    nc = tc.nc
    Wsb = {}
    for wname in ("W1", "W2", "W3"):
        for kc in range(KC):
            t_ = pw.tile([128, D], F32R, tag=f"{wname}_{kc}")
            nc.sync.dma_start(t_[:], ins[wname][kc * 128:(kc + 1) * 128, :])
            Wsb[(wname, kc)] = t_
    for kc in range(KC):
        t_ = pw.tile([128, V], F32R, tag=f"wout_{kc}")
        nc.sync.dma_start(t_[:], ins["Wout"][kc * 128:(kc + 1) * 128, :])
        Wsb[("Wout", kc)] = t_
    if Lr:
        for kc in range(KD // 128):
            t_ = pw.tile([128, D], F32R, tag=f"lw_{kc}")
            nc.sync.dma_start(t_[:], ins["leafWs"][kc * 128:(kc + 1) * 128, :])
            Wsb[("LW", kc)] = t_
        eye4 = pw.tile([128, 32], F32)
        nc.sync.dma_start(eye4[:], ins["eye4"][:])
        Wsb["eye4"] = eye4
    bias_sb = pw.tile([128, 4 * KC], F32)
    nc.sync.dma_start(bias_sb[:], ins["biases"][:])
    Wsb["bias"] = bias_sb
    vrow_sb = pw.tile([1, 2 * D], F32R)
    nc.sync.dma_start(vrow_sb[:], ins["vrow"][:])
    Wsb["vrow"] = vrow_sb
    ident = pw.tile([128, 128], F32)
    make_identity(nc, ident[:])
    Wsb["ident"] = ident
    ones_c = pw.tile([128, 1], F32R)
    ones_cf = pw.tile([128, 1], F32)
    nc.vector.memset(ones_cf[:], 1.0)
    nc.vector.tensor_copy(ones_c[:], ones_cf[:])
    Wsb["ones_c"] = ones_c
    ones_rr = pw.tile([1, 128], F32R)
    nc.vector.memset(ones_rr[:].bitcast(F32), 1.0)
    Wsb["ones_rr"] = ones_rr
    eps_sb = pw.tile([1, 1], F32)
    nc.vector.memset(eps_sb[:], EPS)
    Wsb["eps"] = eps_sb
    return Wsb


@with_exitstack
def _kernel_body(ctx: ExitStack, tc: tile.TileContext, ins, out_d, Wsb, geom,
                 nbody=1):
    R, Lr, KD = geom
    NW = R
    nblk = R // 128
    nb2 = 2 * nbody
    nc = tc.nc
    p_io = ctx.enter_context(tc.tile_pool(name="p_io", bufs=max(2, nbody)))
    p_gth = ctx.enter_context(tc.tile_pool(name="p_gth", bufs=nb2))
    p_act = ctx.enter_context(tc.tile_pool(name="p_act", bufs=nbody))
    p_sm = ctx.enter_context(tc.tile_pool(name="p_sm", bufs=nb2))
    p_tg = ctx.enter_context(tc.tile_pool(name="p_tg", bufs=3))
    p_osb = ctx.enter_context(tc.tile_pool(name="p_osb", bufs=nb2))
    ps_mm = ctx.enter_context(tc.tile_pool(name="ps_mm", bufs=4, space="PSUM"))
    ps_tr = ctx.enter_context(tc.tile_pool(name="ps_tr", bufs=2, space="PSUM"))
    ps_st = ctx.enter_context(tc.tile_pool(name="ps_st", bufs=2, space="PSUM"))

    bias_sb = Wsb["bias"]
    ident = Wsb["ident"]
    st = [dict() for _ in range(nbody)]   # per-body live tiles

    def s_in(b):
        addb = p_io.tile([128, KC * NW], F32, tag="addb", name=f"addb_{b}")
        nc.sync.dma_start(addb[:], ins["memC"][:])
        idx_sb = p_io.tile([128, nblk], I32, tag="idxg", name=f"idxg_{b}")
        nc.sync.dma_start(idx_sb[:], ins["idxg"][:])
        st[b]["addb"], st[b]["idx"] = addb, idx_sb
        if Lr:
            kcl = KD // 128
            lidx_sb = p_io.tile([128, kcl], I32, tag="lidx", name=f"lidx_{b}")
            nc.sync.dma_start(lidx_sb[:], ins["lidx"][:])
            st[b]["lidx"] = lidx_sb

    def s_leaf(b):
        """OL^T [D, Lr] for the tail rows, added into addb."""
        if not Lr:
            return
        kcl = KD // 128
        addb, lidx_sb = st[b]["addb"], st[b]["lidx"]
        lv = p_io.tile([128, kcl * Lr], F32, tag="lv", name=f"lv_{b}")
        for kc in range(kcl):
            gl = p_gth.tile([128, 32], F32, tag="glemb", name=f"gl_{b}_{kc}")
            nc.gpsimd.indirect_dma_start(
                out=gl[:], out_offset=None, in_=ins["lembp"][:],
                in_offset=bass.IndirectOffsetOnAxis(
                    ap=lidx_sb[:, kc:kc + 1], axis=0))
            ptr = ps_tr.tile([128, 512], F32, space="PSUM", tag="ptr",
                             name=f"lptr_{b}_{kc}")
            for j in range(4):
                nc.tensor.matmul(
                    ptr[32 * j:32 * j + 32, 0:Lr],
                    gl[Lr * j:Lr * j + Lr, 0:32],
                    Wsb["eye4"][32 * j:32 * j + 32, 0:Lr],
                    start=True, stop=True, tile_position=(32 * j, 32 * j))
            nc.scalar.activation(lv[:, kc * Lr:(kc + 1) * Lr].bitcast(F32R),
                                 ptr[:, 0:Lr], AF.Identity)
        for mc in range(KC):
            pol = ps_mm.tile([128, NW], F32, space="PSUM", tag="pmm",
                             name=f"pol_{b}_{mc}")
            for kc in range(kcl):
                nc.tensor.matmul(
                    pol[:, 0:Lr], Wsb[("LW", kc)][:, mc * 128:(mc + 1) * 128],
                    lv[:, kc * Lr:(kc + 1) * Lr].bitcast(F32R),
                    start=(kc == 0), stop=(kc == kcl - 1))
            olsb = p_gth.tile([128, Lr], F32, tag="olsb", name=f"ol_{b}_{mc}")
            nc.scalar.activation(
                olsb[:], pol[:, 0:Lr], AF.Identity,
                bias=bias_sb[:, 3 * KC + mc:3 * KC + mc + 1])
            tl = slice(mc * NW + NW - Lr, (mc + 1) * NW)
            nc.vector.tensor_add(addb[:, tl], addb[:, tl], olsb[:])

    def s_gather(b):
        """Gather + transpose embeddings (feature-major embT)."""
        embT = p_act.tile([128, KC * NW], F32, tag="embT", name=f"embT_{b}")
        idx_sb = st[b]["idx"]
        for j in range(nblk):
            gth = p_gth.tile([128, D], F32, tag="gth", name=f"gth_{b}_{j}")
            nc.gpsimd.indirect_dma_start(
                out=gth[:], out_offset=None, in_=ins["femb"][:],
                in_offset=bass.IndirectOffsetOnAxis(
                    ap=idx_sb[:, j:j + 1], axis=0))
            for grp, glen in ((0, 4), (4, 2)):
                ptr = ps_tr.tile([128, 512], F32, space="PSUM", tag="ptr",
                                 name=f"ptr_{b}_{j}_{grp}")
                for ki in range(glen):
                    nc.tensor.transpose(
                        ptr[:, ki * 128:(ki + 1) * 128],
                        gth[:, (grp + ki) * 128:(grp + ki + 1) * 128],
                        ident[:])
                dst = (embT[:].rearrange("p (k w) -> p k w", w=NW)
                       [:, grp:grp + glen, j * 128:(j + 1) * 128])
                nc.vector.tensor_copy(dst.bitcast(F32R), ptr[:, 0:glen * 128])
        st[b]["embT"] = embT

    def fused_layer(b, src, dst, wname, bias_col, A_=None, m_=None, vcol=None):
        """dst = gelu(W^T src [*A - v (x) m] + b); LN applied in psum domain."""
        def mm_group(mc):
            pl = ps_mm.tile([128, NW], F32, space="PSUM", tag="pmm",
                            name=f"pl_{b}_{wname}_{mc}")
            for kc in range(KC):
                nc.tensor.matmul(
                    pl[:], Wsb[(wname, kc)][:, mc * 128:(mc + 1) * 128],
                    src[:, kc * NW:(kc + 1) * NW].bitcast(F32R),
                    start=(kc == 0), stop=(kc == KC - 1 and vcol is None))
            if vcol is not None:
                nc.tensor.matmul(
                    pl[:], Wsb["vrow"][0:1, vcol * D + mc * 128:vcol * D + (mc + 1) * 128],
                    m_[:].bitcast(F32R), start=False, stop=True)
            return pl

        def finish(mc, pl, Asb):
            sl = slice(mc * NW, (mc + 1) * NW)
            if Asb is None:
                nc.scalar.activation(
                    dst[:, sl].bitcast(F32R), pl[:], AF.Gelu,
                    bias=bias_sb[:, bias_col * KC + mc:bias_col * KC + mc + 1])
            else:
                tgc = p_tg.tile([128, NW], F32, tag="tg",
                                name=f"tg_{b}_{wname}_{mc}")
                nc.vector.tensor_mul(tgc[:], pl[:], Asb[:])
                nc.scalar.activation(
                    dst[:, sl].bitcast(F32R), tgc[:], AF.Gelu,
                    bias=bias_sb[:, bias_col * KC + mc:bias_col * KC + mc + 1])

        if vcol is None:
            for mc in range(KC):
                finish(mc, mm_group(mc), None)
            return
        pls = [mm_group(mc) for mc in range(3)]
        pA = ps_mm.tile([128, NW], F32, space="PSUM", tag="pmm",
                        name=f"pA_{b}_{wname}")
        nc.tensor.matmul(pA[:], Wsb["ones_rr"][0:1, :], A_[:].bitcast(F32R),
                         start=True, stop=True)
        Asb = p_sm.tile([128, NW], F32, tag="Asb", name=f"As_{b}_{wname}")
        nc.scalar.activation(Asb[:], pA[:], AF.Identity)
        for i in range(3):
            finish(i, pls[i], Asb)
            pls.append(mm_group(3 + i))
        for i in range(3, KC):
            finish(i, pls[i], Asb)

    def ln_stats(b, src, sq, tagsfx):
        """LN stats: (A_ = rstd row, m_ = mean row), both [1, NW]."""
        for mc in range(KC):
            nc.vector.tensor_mul(sq[:, mc * NW:(mc + 1) * NW].bitcast(F32R),
                                 src[:, mc * NW:(mc + 1) * NW],
                                 src[:, mc * NW:(mc + 1) * NW])
        pss = ps_st.tile([1, NW], F32, space="PSUM", tag="pst",
                         name=f"pss_{b}_{tagsfx}")
        for kc in range(KC):
            nc.tensor.matmul(pss[0:1, :], Wsb["ones_c"][:, 0:1],
                             src[:, kc * NW:(kc + 1) * NW].bitcast(F32R),
                             start=(kc == 0), stop=(kc == KC - 1))
        psq = ps_st.tile([1, NW], F32, space="PSUM", tag="pst",
                         name=f"psq_{b}_{tagsfx}")
        for kc in range(KC):
            nc.tensor.matmul(psq[0:1, :], Wsb["ones_c"][:, 0:1],
                             sq[:, kc * NW:(kc + 1) * NW].bitcast(F32R),
                             start=(kc == 0), stop=(kc == KC - 1))
        m = p_sm.tile([1, NW], F32, tag="m", name=f"m_{b}_{tagsfx}")
        nc.vector.tensor_scalar(out=m[:].bitcast(F32R), in0=pss[0:1, :],
                                scalar1=1.0 / D, scalar2=None, op0=ALU.mult)
        msq = p_sm.tile([1, NW], F32, tag="msq", name=f"msq_{b}_{tagsfx}",
                        bufs=nbody)
        nc.vector.tensor_mul(msq[:], m[:], m[:])
        v = p_sm.tile([1, NW], F32, tag="v", name=f"v_{b}_{tagsfx}",
                      bufs=nbody)
        nc.vector.scalar_tensor_tensor(out=v[:], in0=psq[0:1, :], scalar=1.0 / D,
                                       in1=msq[:], op0=ALU.mult, op1=ALU.subtract)
        A_ = p_sm.tile([1, NW], F32, tag="A", name=f"A_{b}_{tagsfx}")
        with nc.allow_low_precision(reason="fp32r rounding of LN rstd"):
            sd = p_sm.tile([1, NW], F32, tag="sd", name=f"sd_{b}_{tagsfx}",
                           bufs=nbody)
            nc.scalar.activation(sd[:], v[:], AF.Sqrt,
                                 bias=Wsb["eps"][0:1, 0:1])
            nc.vector.reciprocal(A_[:].bitcast(F32R), sd[:])
        return A_, m

    def s_w1(b):
        h = p_act.tile([128, KC * NW], F32, tag="h", name=f"h_{b}")
        fused_layer(b, st[b]["embT"], h, "W1", 0)
        addb = st[b]["addb"]
        for mc in range(KC):
            sl = slice(mc * NW, (mc + 1) * NW)
            nc.vector.tensor_add(h[:, sl].bitcast(F32R), h[:, sl], addb[:, sl])
        st[b]["h"] = h

    def s_ln1(b):
        # sq scratch reuses the embT ring: embT is dead once W1's matmuls read it
        sq = p_act.tile([128, KC * NW], F32, tag="embT", name=f"sq1_{b}")
        st[b]["A1"], st[b]["m1"] = ln_stats(b, st[b]["h"], sq, "a")

    def s_w2(b):
        x2 = p_act.tile([128, KC * NW], F32, tag="x2", name=f"x2_{b}")
        fused_layer(b, st[b]["h"], x2, "W2", 1,
                    A_=st[b]["A1"], m_=st[b]["m1"], vcol=0)
        st[b]["x2"] = x2

    def s_ln2(b):
        sq = p_act.tile([128, KC * NW], F32, tag="embT", name=f"sq2_{b}")
        st[b]["A2"], st[b]["m2"] = ln_stats(b, st[b]["x2"], sq, "b")

    def s_w3(b):
        # x3 reuses the h ring: h is dead once W2's matmuls read it
        x3 = p_act.tile([128, KC * NW], F32, tag="h", name=f"x3_{b}")
        fused_layer(b, st[b]["x2"], x3, "W3", 2,
                    A_=st[b]["A2"], m_=st[b]["m2"], vcol=1)
        st[b]["x3"] = x3

    def s_out(b):
        po = ps_mm.tile([V, NW], F32, space="PSUM", tag="pmm", name=f"po_{b}")
        x3 = st[b]["x3"]
        for kc in range(KC):
            nc.tensor.matmul(po[:], Wsb[("Wout", kc)][:],
                             x3[:, kc * NW:(kc + 1) * NW].bitcast(F32R),
                             start=(kc == 0), stop=(kc == KC - 1))
        eT = p_act.tile([V, NW], F32, tag="eT", name=f"eT_{b}")
        nc.scalar.activation(eT[:], po[:], AF.Exp)
        for j in range(nblk):
            pt = ps_st.tile([128, V], F32, space="PSUM", tag="pst",
                            name=f"pt_{b}_{j}")
            nc.tensor.transpose(pt[:], eT[0:V, j * 128:(j + 1) * 128],
                                ident[0:V, 0:V])
            ssum = p_sm.tile([128, 1], F32, tag="ssum", name=f"ss_{b}_{j}")
            nc.vector.reduce_sum(ssum[:], pt[:], axis=mybir.AxisListType.X)
            rm = p_sm.tile([128, 1], F32, tag="rm", name=f"rm_{b}_{j}")
            nc.vector.reciprocal(rm[:], ssum[:])
            osb = p_osb.tile([128, V], F32, tag="osb", name=f"osb_{b}_{j}")
            nc.vector.tensor_scalar(out=osb[:], in0=pt[:], scalar1=rm[:],
                                    scalar2=None, op0=ALU.mult)
            nc.sync.dma_start(out_d[j * 128:(j + 1) * 128, :], osb[:])

    stages = [s_in, s_leaf, s_gather, s_w1, s_ln1, s_w2, s_ln2, s_w3, s_out]
    for stage in stages:
        for b in range(nbody):
            stage(b)


def _host_prep(inputs):
    """Pure index/layout prep: existence mask, compaction plan, weight
    folding. Returns (geom, in_maps, scatter) for the device run."""
    mem = np.asarray(inputs["memory"], np.float32)
    seqlen = np.asarray(inputs["seq_length"])
    tgt = np.asarray(inputs["tgt"])
    fidx = np.asarray(inputs["feat_idx"])
    femb = np.ascontiguousarray(np.asarray(inputs["feat_embs"], np.float32))
    W1 = np.ascontiguousarray(np.asarray(inputs["W1"], np.float32))
    ln_g = np.asarray(inputs["ln_g"], np.float32)
    ln_b = np.asarray(inputs["ln_b"], np.float32)
    W2 = np.asarray(inputs["W2"], np.float32)
    W3 = np.asarray(inputs["W3"], np.float32)
    b1 = np.asarray(inputs["b1"], np.float32)
    b2 = np.asarray(inputs["b2"], np.float32)
    b3 = np.asarray(inputs["b3"], np.float32)
    Wout = np.ascontiguousarray(np.asarray(inputs["Wout"], np.float32))
    lemb = np.ascontiguousarray(np.asarray(inputs["leaf_emb"], np.float32))
    lW = np.asarray(inputs["leaf_W"], np.float32)
    lb = np.asarray(inputs["leaf_b"], np.float32)

    W2f = np.ascontiguousarray(ln_g[:, None] * W2)
    W3f = np.ascontiguousarray(ln_g[:, None] * W3)
    b2f = (b2 + ln_b @ W2).astype(np.float32)
    b3f = (b3 + ln_b @ W3).astype(np.float32)

    tok_valid = np.arange(S)[None, :] < seqlen[:, None]
    is_slash = (tgt == 0) | (tgt == 1)
    ex = np.zeros((B, S, NN), bool)
    ex[:, :, 0] = tok_valid
    for i in range(1, NN):
        p = (i - 1) // 2
        ex[:, :, i] = ex[:, :, p] & is_slash[:, :, p]

    # compaction: per core, live rows; d>0 rows at the tail
    depth_of = np.zeros(NN, np.int64)
    for d in range(MAXD):
        depth_of[2 ** d - 1:2 ** (d + 1) - 1] = d
    rows_c, tails_c = [], []
    for c in range(NCORES):
        bsl = ex[c * BL:(c + 1) * BL]          # [BL,S,NN]
        bb, ss, nn_ = np.nonzero(bsl)
        dd = depth_of[nn_]
        order = np.argsort(dd > 0, kind="stable")
        bb, ss, nn_, dd = bb[order], ss[order], nn_[order], dd[order]
        head = [(int(b_), int(s_), int(n_)) for b_, s_, n_, d_ in
                zip(bb, ss, nn_, dd) if d_ == 0]
        tail = [(int(b_), int(s_), int(n_), int(d_)) for b_, s_, n_, d_ in
                zip(bb, ss, nn_, dd) if d_ > 0]
        rows_c.append(head)
        tails_c.append(tail)

    maxlive = max(len(h) + len(t) for h, t in zip(rows_c, tails_c))
    maxtail = max(len(t) for t in tails_c)
    maxd_live = max((t[3] for tl in tails_c for t in tl), default=0)
    Lr = 32 if maxtail else 0
    assert maxtail <= Lr, f"leaf budget overflow: {maxtail}"
    R = max(256, -(-maxlive // 128) * 128)
    assert maxlive + (1 if Lr else 0) * 0 <= R and R - Lr >= maxlive - maxtail

    # leaf slot union across live depths: (off n, leaf slot l) l < 2^(d-1)
    maxcnt = 2 ** (maxd_live - 1) if maxd_live else 0
    slots = [(n, l) for n in range(NOFF) for l in range(maxcnt)]
    while len(slots) % 4:
        slots.append(None)
    KD = len(slots) * 32
    geom = (R, Lr, KD)

    biases = np.stack([b1.reshape(KC, 128), b2f.reshape(KC, 128),
                       b3f.reshape(KC, 128), lb.reshape(KC, 128)])
    biases_sb = np.ascontiguousarray(biases.reshape(4 * KC, 128).T)
    vrow = np.concatenate([-W2f.sum(0), -W3f.sum(0)]).reshape(1, 2 * D).astype(np.float32)
    shared = dict(W1=W1, W2=W2f, W3=W3f, Wout=Wout, biases=biases_sb,
                  vrow=vrow, femb=femb)
    if Lr:
        lembp = np.concatenate([lemb, np.zeros((1, 32), np.float32)])
        # leafW rows for slot (n,l): flat rows ((n*LSLOT)+l)*32 ... +32
        lWs = np.zeros((KD, D), np.float32)
        for i, sl_ in enumerate(slots):
            if sl_ is None:
                continue
            n, l = sl_
            r0 = (n * LSLOT + l) * 32
            lWs[i * 32:(i + 1) * 32] = lW[r0:r0 + 32]
        shared.update(lembp=lembp, leafWs=np.ascontiguousarray(lWs),
                      eye4=np.ascontiguousarray(
                          np.tile(np.eye(32, dtype=np.float32), (4, 1))))

    in_maps, scatter = [], []
    tgt_p = np.pad(tgt, ((0, 0), (LC, LC), (0, 0)))          # [B,S+6,NN-ish]
    ex_p = np.pad(ex, ((0, 0), (LC, LC), (0, 0)))
    for c in range(NCORES):
        head, tail = rows_c[c], tails_c[c]
        n_h, n_t = len(head), len(tail)
        rows = list(head) + [(0, 0, 0)] * (R - Lr - n_h) if Lr else list(head)
        if Lr:
            rows += [(b_, s_, n_) for b_, s_, n_, _ in tail]
            rows += [(0, 0, 0)] * (Lr - n_t)
        else:
            rows += [(0, 0, 0)] * (R - n_h)
        assert len(rows) == R
        ridx = np.array([fidx[c * BL + b_, s_, n_] for b_, s_, n_ in rows],
                        np.int32)
        idxg = np.ascontiguousarray(ridx.reshape(R // 128, 128).T)
        memC_rows = np.zeros((R, D), np.float32)
        for i, (b_, s_, n_) in enumerate(rows):
            if i < n_h or (Lr and R - Lr <= i < R - Lr + n_t):
                memC_rows[i] = mem[c * BL + b_, s_]
        memC = np.ascontiguousarray(
            memC_rows.T.reshape(KC, 128, R).transpose(1, 0, 2)
            .reshape(128, KC * R))
        imap = dict(memC=memC, idxg=idxg, **shared)
        if Lr:
            # labels for tail leaf-row j, slot (n,l): depth d row at (b,s):
            # neighbor token s+off, tree slot a+l with a=2^(d-1)-1; masked ->
            # row V (zeros). Mask = ex at that node & valid l < cnt.
            lab = np.full((len(slots), Lr), V, np.int32)
            for j, (b_, s_, n_, d_) in enumerate(tail):
                a, cnt = 2 ** (d_ - 1) - 1, 2 ** (d_ - 1)
                gb = c * BL + b_
                for i, sl_ in enumerate(slots):
                    if sl_ is None:
                        continue
                    n_off, l = sl_
                    if l >= cnt:
                        continue
                    sp = s_ + LC + OFFS[n_off]
                    if ex_p[gb, sp, a + l]:
                        lab[i, j] = tgt_p[gb, sp, a + l]
            # gather order: chunk kc covers slots 4kc..4kc+4; partition
            # p = 32*slot_local + l
            kcl = KD // 128
            lidx = np.zeros((128, kcl), np.int32)
            for kc in range(kcl):
                for jloc in range(4):
                    lidx[32 * jloc:32 * jloc + 32, kc] = lab[4 * kc + jloc]
            imap.update(lidx=np.ascontiguousarray(lidx))
        in_maps.append(imap)
        scatter.append((rows, n_h, n_t))
    return geom, in_maps, scatter


def kernel(**inputs):
    geom, in_maps, scatter = _host_prep(inputs)
    if geom not in _CACHE:
        _CACHE[geom] = _build_nc(geom)
    nc = _CACHE[geom]
    res = run_bass_kernel_spmd(nc, in_maps, core_ids=list(range(NCORES)))
    R, Lr, _ = geom
    out = np.zeros((B, S, NSLOT, V), np.float32)
    for c in range(NCORES):
        dev = res.results[c]["out"]                      # [R, V]
        rows, n_h, n_t = scatter[c]
        for i in range(n_h):
            b_, s_, n_ = rows[i]
            out[c * BL + b_, s_, n_] = dev[i]
        for j in range(n_t):
            i = R - Lr + j
            b_, s_, n_ = rows[i]
            out[c * BL + b_, s_, n_] = dev[i]
    return out
